# revision 20
# baseline (speedup 1.0000x reference)
"""Trainium2 Bass kernel for the Bengio03 Highway BiLM problem.

Math (see reference): L=3 layers, each with fwd/bwd chains. Per (layer, dir):
  padded = [front_pads(4), seq(512), back_pads(4)]          # [B, 520, H]
  pre[t] = sum_{k=0..4} padded[t + k + off] @ W[k*H:(k+1)*H]  (off=0 fwd, 4 bwd)
  x0 = relu(pre + b)
  2x highway: proj = x @ Ws[j] + bs[j]; nonlin,gate = split(proj)
              x = sigmoid(gate)*x + (1-sigmoid(gate))*relu(nonlin)
  out[l,:,:, 0:512] = f chain, [512:1024] = b chain

Implementation strategy (per core, data-parallel over batch: 4 seqs/core):
  - Activations feature-major in SBUF. Layer-0 conv + highway matmuls in
    bf16 (same PE column rate as fp32r, but LDWEIGHTS gets FWL and halves DMA).
  - Layer-1/2 convs (and optionally layer-2 highways) in fp8e4 DoubleRow
    (2 MACs/cycle): weights packed as [128, 2, H] plane pairs (adjacent
    128-feature chunks of the same tap); activations as [128, 2, 528]
    plane-pair tiles (stride 528 % 16 == 0). Per-matrix power-of-2 weight
    scales; descale folded into the ScalarE activation (scale operand).
  - Output written feature-major ([L, BLOC, 2H, S] bf16, contiguous DMA);
    the host transposes back. End-to-end rel err ~1.3-1.7e-2 (gate 2e-2).
"""

import os
import sys

sys.path.insert(0, "/opt/trn_rl_repo")

import numpy as np
import ml_dtypes

import concourse.bass as bass
import concourse.bacc as bacc
import concourse.tile as tile
from concourse import mybir
from concourse.bass_utils import run_bass_kernel_spmd

# Problem constants (hardcoded per spec).
L = 3
WIDTH = 4
H = 512
B = 32
S = 512
NHW = 2
CIN = (WIDTH + 1) * H  # 2560
NCORES = 8
BLOC = B // NCORES  # 4 sequences per core
HC = H // 128  # 4 hidden chunks of 128
SPAD = S + 2 * WIDTH  # 520
SPAD8 = 528  # fp8 plane stride (multiple of 16 bytes)
NT = S // 128  # 4 token tiles of 128
F32 = mybir.dt.float32
BF16 = mybir.dt.bfloat16
F8 = mybir.dt.float8e4
RELU = mybir.ActivationFunctionType.Relu
SIGM = mybir.ActivationFunctionType.Sigmoid
ADD = mybir.AluOpType.add
MAX = mybir.AluOpType.max
DR = mybir.MatmulPerfMode.DoubleRow

# layer-2 highway in fp8 DoubleRow as well (rel err ~1.65e-2 vs 1.33e-2)
HW2_FP8 = True

NP_BF16 = ml_dtypes.bfloat16
NP_F8 = ml_dtypes.float8_e4m3fn


def _build_program():
    nc = bacc.Bacc(
        "TRN2",
        target_bir_lowering=False,
        debug=False,
        enable_asserts=False,
        num_devices=1,
    )

    x_d = nc.dram_tensor("x", [BLOC, S, H], BF16, kind="ExternalInput").ap()
    # conv layer-0 weights, bf16 [CIN, H] per dir
    cw0f_d = nc.dram_tensor("cw0f", [CIN, H], BF16, kind="ExternalInput").ap()
    cw0b_d = nc.dram_tensor("cw0b", [CIN, H], BF16, kind="ExternalInput").ap()
    # conv layer-1/2 weights fp8 DoubleRow packs: [li, rr, 128, 2*H]
    cw8f_d = nc.dram_tensor("cw8f", [2, 10, 128, 2 * H], F8, kind="ExternalInput").ap()
    cw8b_d = nc.dram_tensor("cw8b", [2, 10, 128, 2 * H], F8, kind="ExternalInput").ap()
    # highway weights bf16 (layers 0..HWL_BF-1)
    fhw_d = nc.dram_tensor("fhw", [L, NHW, H, 2 * H], BF16, kind="ExternalInput").ap()
    bhw_d = nc.dram_tensor("bhw", [L, NHW, H, 2 * H], BF16, kind="ExternalInput").ap()
    # layer-2 highway weights fp8 DR: [j, half(nl/gt), hh, 128, 2*H]
    fhw8_d = nc.dram_tensor("fhw8", [NHW, 2, 2, 128, 2 * H], F8, kind="ExternalInput").ap()
    bhw8_d = nc.dram_tensor("bhw8", [NHW, 2, 2, 128, 2 * H], F8, kind="ExternalInput").ap()
    # biases (transposed to [128, chunks]) fp32
    fbt_d = nc.dram_tensor("fbt", [L, 128, HC], F32, kind="ExternalInput").ap()
    bbt_d = nc.dram_tensor("bbt", [L, 128, HC], F32, kind="ExternalInput").ap()
    fhbt_d = nc.dram_tensor("fhbt", [L, NHW, 128, 2 * HC], F32, kind="ExternalInput").ap()
    bhbt_d = nc.dram_tensor("bhbt", [L, NHW, 128, 2 * HC], F32, kind="ExternalInput").ap()
    # descale factors: cols 0-3 conv (dir*2+li); cols 4-11 hw l2 (dir*4+j*2+half)
    csc_d = nc.dram_tensor("csc", [128, 12], F32, kind="ExternalInput").ap()
    # pads: layer-0 bf16 [H, 8] (front||back); layers 1,2 fp8 [2, H, 8]
    pad0_d = nc.dram_tensor("pad0", [H, 8], BF16, kind="ExternalInput").ap()
    pad8_d = nc.dram_tensor("pad8", [2, H, 8], F8, kind="ExternalInput").ap()
    id_d = nc.dram_tensor("ident", [128, 128], BF16, kind="ExternalInput").ap()
    # output feature-major: rows 0:H = f chain, H:2H = b chain
    out_d = nc.dram_tensor("out", [L, BLOC, 2 * H, S], BF16, kind="ExternalOutput").ap()

    with tile.TileContext(nc) as tc:
        with (
            tc.tile_pool(name="consts", bufs=1) as consts,
            tc.tile_pool(name="acts", bufs=16) as acts,
            tc.tile_pool(name="raws", bufs=8) as raws_pool,
            tc.tile_pool(name="a8", bufs=16) as a8pool,
            tc.tile_pool(name="h8", bufs=16) as h8pool,
            tc.tile_pool(name="cw0", bufs=5) as cw0pool,
            tc.tile_pool(name="cw8", bufs=20) as cw8pool,
            tc.tile_pool(name="hww", bufs=4) as hwwpool,
            tc.tile_pool(name="hw8", bufs=16) as hw8pool,
            tc.tile_pool(name="xmid", bufs=12) as xmid,
            tc.tile_pool(name="work", bufs=3) as work,
            tc.tile_pool(name="psum", bufs=2, space="PSUM") as psum,
        ):
            # ---- hot-path constants / weights first (parallel queues) ----
            ident = consts.tile([128, 128], BF16, name="identc", tag="ident", bufs=1)
            nc.sync.dma_start(ident[:], id_d[:, :])

            pads0 = {}
            for c in range(HC):
                pt = consts.tile([128, 8], BF16, name=f"p0_{c}", tag="pads0", bufs=HC)
                nc.scalar.dma_start(pt[:], pad0_d[c * 128:(c + 1) * 128, :])
                pads0[c] = pt

            loaded_cw0 = {}
            loaded_cw8 = {}
            loaded_hw = {}
            loaded_hw8 = {}

            CW0_QUEUES = {0: "s", 3: "s", 6: "s", 9: "s", 1: "a", 4: "a", 7: "a",
                          2: "g", 5: "g", 8: "g"}
            # iteration order matched to expected DMA arrival order at startup
            CW0_ORDER = [0, 3, 1, 2, 6, 4, 5, 9, 7, 8]

            def ensure_cw0(dirc):
                # layer-0 conv weights: 10 tiles [128, 2, 512] bf16 (r = 2*tt + q)
                if dirc not in loaded_cw0:
                    src = cw0f_d if dirc == "f" else cw0b_d
                    tiles = []
                    for tt in range(10):
                        w = cw0pool.tile(
                            [128, 2, H], BF16, name=f"cw0_{dirc}_{tt}", tag="cw0", bufs=20
                        )
                        eng = {"s": nc.sync, "a": nc.scalar, "g": nc.gpsimd}[CW0_QUEUES[tt]]
                        eng.dma_start(
                            w[:],
                            src[tt * 256:(tt + 1) * 256, :].rearrange(
                                "(q p) h -> p q h", p=128
                            ),
                        )
                        tiles.append(w)
                    loaded_cw0[dirc] = tiles
                return loaded_cw0[dirc]

            def ensure_cw8(dirc, l):
                # fp8 DR conv weights for layer l in {1,2}: 10 tiles [128, 2, 512]
                li = l - 1
                if (dirc, li) not in loaded_cw8:
                    src = cw8f_d if dirc == "f" else cw8b_d
                    tiles = []
                    for rr in range(10):
                        w = cw8pool.tile(
                            [128, 2, H], F8, name=f"cw8_{dirc}{li}_{rr}", tag="cw8", bufs=20
                        )
                        nc.sync.dma_start(
                            w[:],
                            src[li, rr].rearrange("p (i h) -> p i h", i=2),
                        )
                        tiles.append(w)
                    loaded_cw8[(dirc, li)] = tiles
                return loaded_cw8[(dirc, li)]

            def ensure_hww(dirc, l):
                # bf16 highway weights: per j one tile [128, 4, 1024]
                if (dirc, l) not in loaded_hw:
                    src = fhw_d if dirc == "f" else bhw_d
                    res = []
                    for j in range(NHW):
                        w = hwwpool.tile(
                            [128, 4, 2 * H], BF16, name=f"hw_{dirc}{l}_{j}", tag="hww", bufs=4
                        )
                        nc.scalar.dma_start(
                            w[:],
                            src[l, j].rearrange("(h p) c -> p h c", p=128),
                        )
                        res.append(w)
                    loaded_hw[(dirc, l)] = res
                return loaded_hw[(dirc, l)]

            def ensure_hw8(dirc):
                # fp8 DR layer-2 highway weights: tiles [(j, half, hh)] [128, 2, H]
                if dirc not in loaded_hw8:
                    src = fhw8_d if dirc == "f" else bhw8_d
                    res = {}
                    for j in range(NHW):
                        for half in range(2):
                            for hh in range(2):
                                w = hw8pool.tile(
                                    [128, 2, H], F8,
                                    name=f"hw8_{dirc}{j}_{half}_{hh}", tag="hw8", bufs=16,
                                )
                                nc.gpsimd.dma_start(
                                    w[:],
                                    src[j, half, hh].rearrange("p (i h) -> p i h", i=2),
                                )
                                res[(j, half, hh)] = w
                    loaded_hw8[dirc] = res
                return loaded_hw8[dirc]

            # ---- input stage: load + transpose to feature-major bf16 ----
            xT = {}
            raws = {}

            def load_raws(b, eng):
                for t4 in range(NT):
                    raw = raws_pool.tile([128, H], BF16, name=f"inraw_{b}_{t4}", tag="raws", bufs=8)
                    eng.dma_start(raw[:], x_d[b, t4 * 128:(t4 + 1) * 128, :])
                    raws[(b, t4)] = raw

            load_raws(1, nc.gpsimd)
            load_raws(0, nc.scalar)
            ensure_cw0("f")
            load_raws(2, nc.scalar)
            load_raws(3, nc.scalar)
            for b in range(BLOC):
                for c in range(HC):
                    at = acts.tile([128, SPAD], BF16, name=f"xT_{c}_{b}", tag="acts", bufs=16)
                    tp = psum.tile([128, S], BF16, name=f"tpi_{c}_{b}", tag="hpsum", bufs=5)
                    for t4 in range(NT):
                        nc.tensor.matmul(
                            tp[:, t4 * 128:(t4 + 1) * 128],
                            lhsT=raws[(b, t4)][:, c * 128:(c + 1) * 128],
                            rhs=ident[:],
                            is_transpose=True,
                        )
                    nc.scalar.copy(at[:, WIDTH:WIDTH + S], tp[:])
                    nc.vector.tensor_copy(at[:, 0:WIDTH], pads0[c][:, 0:WIDTH])
                    nc.vector.tensor_copy(at[:, WIDTH + S:SPAD], pads0[c][:, WIDTH:])
                    xT[(c, b)] = at

            # ---- remaining constants (cold path) ----
            pads8 = {}
            for li in range(2):
                for c in range(HC):
                    pt = consts.tile([128, 8], F8, name=f"p8_{li}_{c}", tag="pads8", bufs=2 * HC)
                    nc.sync.dma_start(pt[:], pad8_d[li, c * 128:(c + 1) * 128, :])
                    pads8[(li, c)] = pt
            cbias = {}
            for dirc, src in (("f", fbt_d), ("b", bbt_d)):
                for l in range(L):
                    t = consts.tile([128, HC], F32, name=f"cb_{dirc}{l}", tag="cb", bufs=2 * L)
                    nc.sync.dma_start(t[:], src[l])
                    cbias[(dirc, l)] = t
            cscale = consts.tile([128, 12], F32, name="cscale", tag="csc", bufs=1)
            nc.sync.dma_start(cscale[:], csc_d[:, :])
            hbias = {}
            for dirc, src in (("f", fhbt_d), ("b", bhbt_d)):
                for l in range(L):
                    for j in range(NHW):
                        t = consts.tile(
                            [128, 2 * HC], F32, name=f"hb_{dirc}{l}_{j}", tag="hb",
                            bufs=2 * L * NHW,
                        )
                        nc.sync.dma_start(t[:], src[l, j])
                        hbias[(dirc, l, j)] = t

            # ---- stages ----
            def conv0_stage(dirc, pair, wtiles):
                # layer-0 conv from bf16 xT tiles
                off0 = 0 if dirc == "f" else WIDTH
                x0 = {}
                for b in pair:
                    for n in range(HC):
                        ps = psum.tile([128, S], F32, name=f"cps_{b}_{n}", tag="cpsum", bufs=3)
                        for i, tt in enumerate(CW0_ORDER):
                            for q in range(2):
                                r = 2 * tt + q
                                k, ci = divmod(r, HC)
                                off = off0 + k
                                nc.tensor.matmul(
                                    ps[:],
                                    lhsT=wtiles[tt][:, q, n * 128:(n + 1) * 128],
                                    rhs=xT[(ci, b)][:, off:off + S],
                                    start=(i == 0 and q == 0),
                                    stop=(i == 9 and q == 1),
                                )
                        xt = xmid.tile([128, S], BF16, name=f"x0_{b}_{n}", tag="x0", bufs=12)
                        nc.vector.tensor_scalar(
                            xt[:], ps[:], cbias[(dirc, 0)][:, n:n + 1], 0.0, ADD, MAX
                        )
                        x0[(n, b)] = xt
                return x0

            def conv8_stage(dirc, l, srcset, pair, wtiles):
                # fp8 DoubleRow conv for layers 1,2; srcset: fp8 pair tiles (cc, b)
                off0 = 0 if dirc == "f" else WIDTH
                li = l - 1
                sci = (0 if dirc == "f" else 2) + li
                sc = cscale[:, sci:sci + 1]
                hw8 = HW2_FP8 and l == 2
                x0 = {}
                x0f8 = {}
                for b in pair:
                    for n in range(HC):
                        ps = psum.tile([128, S], F32, name=f"cps_{b}_{n}", tag="cpsum", bufs=3)
                        for rr in range(10):
                            k, cc = divmod(rr, 2)
                            off = off0 + k
                            nc.tensor.matmul(
                                ps[:],
                                lhsT=wtiles[rr][:, :, n * 128:(n + 1) * 128],
                                rhs=srcset[(cc, b)][:, :, off:off + S],
                                start=(rr == 0),
                                stop=(rr == 9),
                                perf_mode=DR,
                            )
                        xt = xmid.tile([128, S], BF16, name=f"x0_{b}_{n}", tag="x0", bufs=12)
                        nc.scalar.activation(
                            xt[:], ps[:], RELU, bias=cbias[(dirc, l)][:, n:n + 1], scale=sc
                        )
                        x0[(n, b)] = xt
                        if hw8:
                            hh, i = divmod(n, 2)
                            key = (hh, b)
                            if key not in x0f8:
                                x0f8[key] = h8pool.tile(
                                    [128, 2, S], F8, name=f"x08_{b}_{hh}", tag="h8", bufs=16
                                )
                            nc.vector.tensor_copy(x0f8[key][:, i, :], xt[:])
                return (x0, x0f8) if hw8 else x0

            def hw_stage(dirc, l, j, srcset, pair, wt, final):
                hb = hbias[(dirc, l, j)]
                outs = {}
                for b in pair:
                    for c in range(HC):
                        pnl = psum.tile([128, S], F32, name=f"hnl_{b}_{c}", tag="hpsum", bufs=5)
                        for h in range(HC):
                            nc.tensor.matmul(
                                pnl[:],
                                lhsT=wt[:, h, c * 128:(c + 1) * 128],
                                rhs=srcset[(h, b)][:],
                                start=(h == 0),
                                stop=(h == HC - 1),
                            )
                        pgt = psum.tile([128, S], F32, name=f"hgt_{b}_{c}", tag="hpsum", bufs=5)
                        for h in range(HC):
                            nc.tensor.matmul(
                                pgt[:],
                                lhsT=wt[:, h, H + c * 128:H + (c + 1) * 128],
                                rhs=srcset[(h, b)][:],
                                start=(h == 0),
                                stop=(h == HC - 1),
                            )
                        finish_hw(dirc, l, j, b, c, pnl, pgt, srcset, outs, final,
                                  hb[:, c:c + 1], hb[:, HC + c:HC + c + 1], None, None)
                return outs

            def hw8_stage(dirc, j, srcbf, src8, pair, w8, final):
                # layer-2 highway with fp8 DR matmuls
                l = 2
                hb = hbias[(dirc, l, j)]
                sbase = 4 + (0 if dirc == "f" else 4) + j * 2
                sc_nl = cscale[:, sbase:sbase + 1]
                sc_gt = cscale[:, sbase + 1:sbase + 2]
                outs = {}
                for b in pair:
                    for c in range(HC):
                        pnl = psum.tile([128, S], F32, name=f"hnl_{b}_{c}", tag="hpsum", bufs=5)
                        for hh in range(2):
                            nc.tensor.matmul(
                                pnl[:],
                                lhsT=w8[(j, 0, hh)][:, :, c * 128:(c + 1) * 128],
                                rhs=src8[(hh, b)][:],
                                start=(hh == 0),
                                stop=(hh == 1),
                                perf_mode=DR,
                            )
                        pgt = psum.tile([128, S], F32, name=f"hgt_{b}_{c}", tag="hpsum", bufs=5)
                        for hh in range(2):
                            nc.tensor.matmul(
                                pgt[:],
                                lhsT=w8[(j, 1, hh)][:, :, c * 128:(c + 1) * 128],
                                rhs=src8[(hh, b)][:],
                                start=(hh == 0),
                                stop=(hh == 1),
                                perf_mode=DR,
                            )
                        finish_hw(dirc, l, j, b, c, pnl, pgt, srcbf, outs, final,
                                  hb[:, c:c + 1], hb[:, HC + c:HC + c + 1], sc_nl, sc_gt)
                return outs

            def finish_hw(dirc, l, j, b, c, pnl, pgt, srcset, outs, final,
                          bnl, bgt, sc_nl, sc_gt):
                hw8next = HW2_FP8 and l == 2 and j == 0
                r = work.tile([128, S], BF16, name=f"r_{b}_{c}", tag="r", bufs=3)
                if sc_nl is None:
                    nc.scalar.activation(r[:], pnl[:], RELU, bias=bnl)
                else:
                    nc.scalar.activation(r[:], pnl[:], RELU, bias=bnl, scale=sc_nl)
                g = work.tile([128, S], BF16, name=f"g_{b}_{c}", tag="g", bufs=3)
                if sc_gt is None:
                    nc.scalar.activation(g[:], pgt[:], SIGM, bias=bgt)
                else:
                    nc.scalar.activation(g[:], pgt[:], SIGM, bias=bgt, scale=sc_gt)
                d = work.tile([128, S], BF16, name=f"d_{b}_{c}", tag="d", bufs=3)
                nc.vector.tensor_sub(d[:], srcset[(c, b)][:], r[:])
                m = work.tile([128, S], BF16, name=f"m_{b}_{c}", tag="m", bufs=4)
                nc.vector.tensor_mul(m[:], g[:], d[:])
                if final:
                    o = work.tile([128, S], BF16, name=f"fin_{b}_{c}", tag="fin", bufs=4)
                    nc.vector.tensor_add(o[:], m[:], r[:])
                    emit_out(dirc, l, o, c, b)
                    if l + 1 < L:
                        cc, i = divmod(c, 2)
                        at8 = ensure_a8(l + 1, dirc, cc, b)
                        nc.vector.tensor_copy(at8[:, i, WIDTH:WIDTH + S], o[:])
                    outs[(c, b)] = o
                else:
                    o = xmid.tile([128, S], BF16, name=f"x1_{b}_{c}", tag="x1", bufs=12)
                    nc.vector.tensor_add(o[:], m[:], r[:])
                    outs[(c, b)] = o
                    if hw8next:
                        hh, i = divmod(c, 2)
                        key = ("x18", l, dirc, hh, b)
                        if key not in x18tiles:
                            x18tiles[key] = h8pool.tile(
                                [128, 2, S], F8, name=f"x18_{b}_{hh}", tag="h8", bufs=16
                            )
                        nc.vector.tensor_copy(x18tiles[key][:, i, :], o[:])

            x18tiles = {}

            # fp8 activation pair tiles for the next layer's conv, pads pre-written
            a8tiles = {}

            def ensure_a8(l, dirc, cc, b):
                key = (l, dirc, cc, b)
                if key not in a8tiles:
                    li = l - 1
                    at8 = a8pool.tile(
                        [128, 2, SPAD8], F8, name=f"a8_{dirc}{l}_{cc}_{b}", tag="a8", bufs=16
                    )
                    for i in range(2):
                        c = 2 * cc + i
                        nc.vector.tensor_copy(at8[:, i, 0:WIDTH], pads8[(li, c)][:, 0:WIDTH])
                        nc.vector.tensor_copy(
                            at8[:, i, WIDTH + S:SPAD], pads8[(li, c)][:, WIDTH:]
                        )
                    a8tiles[key] = at8
                return a8tiles[key]

            def emit_out(dirc, l, o, c, b):
                doff = 0 if dirc == "f" else H
                h0 = doff + c * 128
                eng = nc.gpsimd if (l == 2 and (c + b) % 2 == 0) else nc.sync
                eng.dma_start(out_d[l, b, h0:h0 + 128, :], o[:])

            # ---- main chain: f0 f1 b0 f2 b1 b2 (independent conv work
            # emitted adjacent to each layer-2 tail to fill dependency stalls) ----
            PAIRS = [(0, 1), (2, 3)]
            CHAIN = [("f", 0), ("f", 1), ("b", 0), ("f", 2), ("b", 1), ("b", 2)]
            PREFETCH = {
                ("f", 0): [],
                ("f", 1): [lambda: ensure_cw0("b"), lambda: ensure_hww("b", 0),
                           lambda: ensure_cw8("f", 2), lambda: ensure_hw8("f")],
                ("b", 0): [lambda: ensure_cw8("b", 1), lambda: ensure_hww("b", 1)],
                ("f", 2): [lambda: ensure_hw8("b"), lambda: ensure_cw8("b", 2)],
                ("b", 1): [],
                ("b", 2): [],
            }
            for dirc, l in CHAIN:
                    use_hw8 = HW2_FP8 and l == 2
                    if l == 0:
                        cw = ensure_cw0(dirc)
                    else:
                        cw = ensure_cw8(dirc, l)
                    if use_hw8:
                        w8 = ensure_hw8(dirc)
                    else:
                        hw = ensure_hww(dirc, l)
                    for pf in PREFETCH[(dirc, l)]:
                        pf()
                    if l == 1:
                        pass
                    for pair in PAIRS:
                        if l == 0:
                            x0 = conv0_stage(dirc, pair, cw)
                        else:
                            src8 = {
                                (cc, b): a8tiles[(l, dirc, cc, b)]
                                for cc in range(2)
                                for b in pair
                            }
                            res = conv8_stage(dirc, l, src8, pair, cw)
                            if use_hw8:
                                x0, x0f8 = res
                            else:
                                x0 = res
                        if use_hw8:
                            x1 = hw8_stage(dirc, 0, x0, x0f8, pair, w8, final=False)
                            x18 = {
                                (hh, b): x18tiles[("x18", 2, dirc, hh, b)]
                                for hh in range(2)
                                for b in pair
                            }
                            hw8_stage(dirc, 1, x1, x18, pair, w8, final=True)
                        else:
                            x1 = hw_stage(dirc, l, 0, x0, pair, hw[0], final=False)
                            hw_stage(dirc, l, 1, x1, pair, hw[1], final=True)

    nc.compile()
    return nc


_CACHE = {}


def _get_program():
    if "nc" not in _CACHE:
        _CACHE["nc"] = _build_program()
    return _CACHE["nc"]


def _q8(w, s):
    # quantize to TRN fp8e4 grid (values kept <= 120, identical to OCP e4m3fn)
    return np.asarray(np.clip(w * s, -240.0, 240.0), dtype=np.float32).astype(NP_F8)


def _p2scale(w):
    return 2.0 ** np.floor(np.log2(120.0 / np.abs(w).max()))


def _pack_cw8(W):
    # W: [CIN, H] fp32 -> scale + [10, 128, 2H] fp8 DoubleRow pack
    # rr = k*2 + cc pairs row chunks r0 = k*HC + 2cc, r1 = r0 + 1
    s = _p2scale(W)
    Wq = _q8(W, s)
    out = np.zeros((10, 128, 2 * H), dtype=NP_F8)
    for rr in range(10):
        k, cc = divmod(rr, 2)
        r0 = k * HC + 2 * cc
        out[rr, :, 0:H] = Wq[r0 * 128:(r0 + 1) * 128, :]
        out[rr, :, H:] = Wq[(r0 + 1) * 128:(r0 + 2) * 128, :]
    return s, out


def _pack_hw8(W):
    # W: [NHW, H, 2H] fp32 -> scales [NHW, 2], pack [NHW, 2, 2, 128, 2H] fp8
    scales = np.zeros((NHW, 2), dtype=np.float64)
    out = np.zeros((NHW, 2, 2, 128, 2 * H), dtype=NP_F8)
    for j in range(NHW):
        for half in range(2):
            blk = W[j][:, half * H:(half + 1) * H]  # [H, H]
            s = _p2scale(blk)
            scales[j, half] = s
            q = _q8(blk, s)
            for hh in range(2):
                out[j, half, hh, :, 0:H] = q[(2 * hh) * 128:(2 * hh + 1) * 128, :]
                out[j, half, hh, :, H:] = q[(2 * hh + 1) * 128:(2 * hh + 2) * 128, :]
    return scales, out


def _make_in_maps(inputs):
    x = np.ascontiguousarray(inputs["inputs"], dtype=np.float32).astype(NP_BF16)
    fw = np.asarray(inputs["fwd_W"], dtype=np.float32)
    bw = np.asarray(inputs["bwd_W"], dtype=np.float32)

    cw0f = np.ascontiguousarray(fw[0]).astype(NP_BF16)
    cw0b = np.ascontiguousarray(bw[0]).astype(NP_BF16)
    sf1, f1 = _pack_cw8(fw[1])
    sf2, f2 = _pack_cw8(fw[2])
    sb1, b1 = _pack_cw8(bw[1])
    sb2, b2 = _pack_cw8(bw[2])
    cw8f = np.stack([f1, f2], axis=0)
    cw8b = np.stack([b1, b2], axis=0)

    fhwf = np.asarray(inputs["fwd_hw_W"], dtype=np.float32)
    bhwf = np.asarray(inputs["bwd_hw_W"], dtype=np.float32)
    fhw = fhwf.astype(NP_BF16)
    bhw = bhwf.astype(NP_BF16)
    fsc, fhw8 = _pack_hw8(fhwf[2])
    bsc, bhw8 = _pack_hw8(bhwf[2])

    csc = np.empty((128, 12), dtype=np.float32)
    csc[:, 0] = 1.0 / sf1
    csc[:, 1] = 1.0 / sf2
    csc[:, 2] = 1.0 / sb1
    csc[:, 3] = 1.0 / sb2
    for j in range(NHW):
        for half in range(2):
            csc[:, 4 + j * 2 + half] = 1.0 / fsc[j, half]
            csc[:, 8 + j * 2 + half] = 1.0 / bsc[j, half]

    fbt = np.ascontiguousarray(
        np.asarray(inputs["fwd_b"], dtype=np.float32).reshape(L, HC, 128).transpose(0, 2, 1)
    )
    bbt = np.ascontiguousarray(
        np.asarray(inputs["bwd_b"], dtype=np.float32).reshape(L, HC, 128).transpose(0, 2, 1)
    )
    fhbt = np.ascontiguousarray(
        np.asarray(inputs["fwd_hw_b"], dtype=np.float32)
        .reshape(L, NHW, 2 * HC, 128)
        .transpose(0, 1, 3, 2)
    )
    bhbt = np.ascontiguousarray(
        np.asarray(inputs["bwd_hw_b"], dtype=np.float32)
        .reshape(L, NHW, 2 * HC, 128)
        .transpose(0, 1, 3, 2)
    )
    fp = np.asarray(inputs["fwd_pads"], dtype=np.float32)  # [L, 4, H]
    bp = np.asarray(inputs["bwd_pads"], dtype=np.float32)
    # layer-l pads: front = fwd_pads[l] (cols 0:4), back = bwd_pads[l] (cols 4:8)
    pad0 = np.concatenate([fp[0].T, bp[0].T], axis=1).astype(NP_BF16)  # [H, 8]
    pad8 = np.stack(
        [
            np.concatenate([fp[l].T, bp[l].T], axis=1).astype(NP_F8)
            for l in (1, 2)
        ],
        axis=0,
    )
    ident = np.eye(128, dtype=np.float32).astype(NP_BF16)

    shared = {
        "cw0f": cw0f, "cw0b": cw0b, "cw8f": cw8f, "cw8b": cw8b,
        "fhw": fhw, "bhw": bhw, "fhw8": fhw8, "bhw8": bhw8,
        "fbt": fbt, "bbt": bbt, "fhbt": fhbt, "bhbt": bhbt,
        "csc": csc, "pad0": pad0, "pad8": pad8, "ident": ident,
    }
    in_maps = []
    for i in range(NCORES):
        m = dict(shared)
        m["x"] = np.ascontiguousarray(x[i * BLOC:(i + 1) * BLOC])
        in_maps.append(m)
    return in_maps


def _run(inputs, trace=False, tmpdir=None):
    nc = _get_program()
    in_maps = _make_in_maps(inputs)
    res = run_bass_kernel_spmd(
        nc, in_maps, core_ids=list(range(NCORES)), trace=trace, tmpdir=tmpdir
    )
    # out: [L, BLOC, 2H, S] bf16 feature-major -> [L, B, S, 2H] fp32
    parts = [
        np.asarray(r["out"]).astype(np.float32).transpose(0, 1, 3, 2)
        for r in res.results
    ]
    out = np.concatenate(parts, axis=1)
    return np.ascontiguousarray(out), res


def kernel(**inputs):
    trace = bool(int(os.environ.get("BASS_KERNEL_TRACE", "0")))
    out, _ = _run(inputs, trace=trace)
    return out
